# revision 6
# baseline (speedup 1.0000x reference)
"""ContinuousTimeRNN Trainium2 kernel.

Data-parallel over batch N=512 across 8 NeuronCores (64 rows each).
Per core, per step t (h kept transposed: H on partitions):
    a = relu(tanh(h))                       (ACT + DVE)
    delta = 0.1*(a @ W_rec + x_t @ W_in + bias)   (PE: a-chunks stationary,
                                             pre-scaled W streams; x via K=3
                                             matmul with a ones-row for bias)
    h' = 0.9*h + delta                      (PE transposes delta back to
                                             T-layout, fused DVE update)
    y_t = h' @ W_out                        (batched every 25 steps from the
                                             h history buffer, PSUM->DRAM)
"""

import sys

sys.path.insert(0, "/opt/trn_rl_repo")

import numpy as np

ALPHA = 0.1
T, N, H, DIN, DOUT, INIT = 1000, 512, 512, 2, 2, 2
NCORES = 8
NS = N // NCORES          # 64 batch rows per core
WIN = 25                  # h-history window (steps)
BODY = 2 * WIN            # steps per For_i body (ping-pong windows)
NK = H // 128             # 4 H-chunks


def _build_nc(t_total=T):
    import concourse.bass as bass
    import concourse.mybir as mybir
    from concourse import bacc
    from concourse.tile import TileContext
    from concourse.masks import make_identity
    from concourse.bass import ds

    fp32 = mybir.dt.float32
    nc = bacc.Bacc("TRN2", target_bir_lowering=False, debug=False,
                   num_devices=NCORES)

    # -------- DRAM I/O (per core) --------
    wrec_d = nc.dram_tensor("wrec", [NK, 128, H], fp32, kind="ExternalInput").ap()
    win3_d = nc.dram_tensor("win3", [DIN + 1, H], fp32, kind="ExternalInput").ap()
    wout_d = nc.dram_tensor("wout", [NK, 128, DOUT], fp32, kind="ExternalInput").ap()
    fcw3_d = nc.dram_tensor("fcw3", [INIT + 1, H], fp32, kind="ExternalInput").ap()
    init3_d = nc.dram_tensor("init3", [INIT + 1, NS], fp32, kind="ExternalInput").ap()
    xt_d = nc.dram_tensor("xt", [DIN + 1, t_total * NS], fp32, kind="ExternalInput").ap()
    y_d = nc.dram_tensor("y", [DOUT, t_total * NS], fp32, kind="ExternalOutput").ap()

    with TileContext(nc) as tc:
        with (
            tc.tile_pool(name="wpool", bufs=1) as wpool,
            tc.tile_pool(name="hpool", bufs=1) as hpool,
            tc.tile_pool(name="apool", bufs=2) as apool,
            tc.tile_pool(name="dpool", bufs=2) as dpool,
            tc.tile_pool(name="xpool", bufs=2) as xpool,
            tc.tile_pool(name="p1", bufs=2, space="PSUM") as p1pool,
            tc.tile_pool(name="p3", bufs=2, space="PSUM") as p3pool,
            tc.tile_pool(name="py", bufs=2, space="PSUM") as pypool,
        ):
            # -------- persistent SBUF --------
            wrec_sb = wpool.tile([128, NK, H], fp32)       # 0.1*W_rec chunks
            win3_sb = wpool.tile([DIN + 1, H], fp32)       # 0.1*[W_in; bias]
            wout_sb = wpool.tile([128, NK, DOUT], fp32)    # W_out chunks
            _ = bass  # keep import
            fcw3_sb = wpool.tile([INIT + 1, H], fp32)      # [fc_w.T; fc_b]
            init3_sb = wpool.tile([INIT + 1, NS], fp32)    # [initdir.T; ones]
            ident = wpool.tile([NS, NS], fp32)             # 64x64 identity
            hist_a = hpool.tile([128, NK, WIN * NS], fp32)
            hist_b = hpool.tile([128, NK, WIN * NS], fp32)
            hist = [hist_a, hist_b]

            for k in range(NK):
                nc.sync.dma_start(out=wrec_sb[:, k, :], in_=wrec_d[k])
                nc.sync.dma_start(out=wout_sb[:, k, :], in_=wout_d[k])
            nc.sync.dma_start(out=win3_sb[:], in_=win3_d)
            nc.sync.dma_start(out=fcw3_sb[:], in_=fcw3_d)
            nc.sync.dma_start(out=init3_sb[:], in_=init3_d)
            make_identity(nc, ident[:])

            # -------- h0 = fc(initdir), written to hist[1] slice WIN-1 --------
            ph0 = p3pool.tile([128, NK * NS], fp32)
            for m in range(NK):
                nc.tensor.matmul(ph0[:, m * NS:(m + 1) * NS],
                                 fcw3_sb[:, m * 128:(m + 1) * 128],
                                 init3_sb[:], start=True, stop=True)
            nc.scalar.copy(out=hist[1][:, :, (WIN - 1) * NS: WIN * NS],
                           in_=ph0[:].rearrange("p (k n) -> p k n", k=NK))

            # -------- time loop --------
            with tc.For_i(0, t_total, BODY) as iv:
                xbuf = xpool.tile([DIN + 1, BODY * NS], fp32)
                nc.sync.dma_start(out=xbuf[:], in_=xt_d[:, ds(iv * NS, BODY * NS)])

                for w in range(2):
                    hc, hp = hist[w], hist[1 - w]
                    for s in range(WIN):
                        prev = (hp[:, :, (WIN - 1) * NS: WIN * NS] if s == 0
                                else hc[:, :, (s - 1) * NS: s * NS])
                        cur = hc[:, :, s * NS:(s + 1) * NS]
                        import concourse.mybir as _mb
                        a_t = apool.tile([128, NK * NS], fp32)
                        av = a_t[:].rearrange("p (k n) -> p k n", k=NK)
                        nc.scalar.activation(av, prev,
                                             _mb.ActivationFunctionType.Tanh)
                        nc.vector.tensor_scalar_max(a_t[:], a_t[:], 0.0)

                        psum1 = p1pool.tile([NS, H], fp32)
                        for k in range(NK):
                            nc.tensor.matmul(psum1[:],
                                             a_t[:, k * NS:(k + 1) * NS],
                                             wrec_sb[:, k, :],
                                             start=(k == 0), stop=False)
                        xcol = (w * WIN + s) * NS
                        nc.tensor.matmul(psum1[:], xbuf[:, xcol:xcol + NS],
                                         win3_sb[:], start=False, stop=True)

                        d_t = dpool.tile([NS, H], fp32)
                        nc.scalar.copy(out=d_t[:], in_=psum1[:])

                        psum3 = p3pool.tile([128, NK * NS], fp32)
                        for b in range(NK):
                            nc.tensor.transpose(psum3[:, b * NS:(b + 1) * NS],
                                                d_t[:, b * 128:(b + 1) * 128],
                                                ident[:])
                        nc.vector.scalar_tensor_tensor(
                            out=cur, in0=prev, scalar=1.0 - ALPHA,
                            in1=psum3[:].rearrange("p (k n) -> p k n", k=NK),
                            op0=_mb.AluOpType.mult, op1=_mb.AluOpType.add)

                    # y flush for this window: yT = W_out.T @ hT (batched)
                    QW = WIN * NS // 4  # 400 columns per quarter
                    for q in range(4):
                        py = pypool.tile([DOUT, QW], fp32)
                        for k in range(NK):
                            nc.tensor.matmul(py[:], wout_sb[:, k, :],
                                             hc[:, k, q * QW:(q + 1) * QW],
                                             start=(k == 0), stop=(k == NK - 1))
                        ysb = dpool.tile([DOUT, QW], fp32, tag="ysb")
                        nc.vector.tensor_copy(ysb[:], py[:])
                        nc.sync.dma_start(
                            out=y_d[:, ds(iv * NS + w * WIN * NS + q * QW, QW)],
                            in_=ysb[:])

    nc.compile()
    return nc


_NC_CACHE = {}


def _get_nc():
    if "nc" not in _NC_CACHE:
        _NC_CACHE["nc"] = _build_nc()
    return _NC_CACHE["nc"]


def kernel(initdir, velocities, fc_w, fc_b, W_in, W_rec, W_out, bias):
    from concourse.bass_utils import run_bass_kernel_spmd

    initdir = np.asarray(initdir, np.float32)
    velocities = np.asarray(velocities, np.float32)
    fc_w = np.asarray(fc_w, np.float32)
    fc_b = np.asarray(fc_b, np.float32)
    W_in = np.asarray(W_in, np.float32)
    W_rec = np.asarray(W_rec, np.float32)
    W_out = np.asarray(W_out, np.float32)
    bias = np.asarray(bias, np.float32)

    # host-side weight prep (shared across cores)
    wrec = (ALPHA * W_rec).reshape(NK, 128, H)
    win3 = ALPHA * np.concatenate([W_in, bias[None, :]], axis=0)     # (3, H)
    wout = W_out.reshape(NK, 128, DOUT)
    fcw3 = np.concatenate([fc_w.T, fc_b[None, :]], axis=0)           # (3, H)

    in_maps = []
    for c in range(NCORES):
        sl = slice(c * NS, (c + 1) * NS)
        init3 = np.concatenate([initdir[sl].T,
                                np.ones((1, NS), np.float32)], axis=0)
        # xt[p, t*NS+n] = velocities[t, c*NS+n, p]; row DIN = ones
        xs = velocities[:, sl, :]                                    # (T, NS, 2)
        xt = np.empty((DIN + 1, T * NS), np.float32)
        xt[:DIN] = xs.transpose(2, 0, 1).reshape(DIN, T * NS)
        xt[DIN] = 1.0
        in_maps.append({
            "wrec": np.ascontiguousarray(wrec),
            "win3": np.ascontiguousarray(win3),
            "wout": np.ascontiguousarray(wout),
            "fcw3": np.ascontiguousarray(fcw3),
            "init3": np.ascontiguousarray(init3),
            "xt": xt,
        })

    nc = _get_nc()
    res = run_bass_kernel_spmd(nc, in_maps, list(range(NCORES)))

    out = np.empty((T, N, DOUT), np.float32)
    for c in range(NCORES):
        yt = res.results[c]["y"]                                     # (2, T*NS)
        out[:, c * NS:(c + 1) * NS, :] = (
            yt.reshape(DOUT, T, NS).transpose(1, 2, 0))
    return out


# revision 9
# speedup vs baseline: 47.9043x; 47.9043x over previous
"""ContinuousTimeRNN Trainium2 kernel.

Data-parallel over batch N=512 across 8 NeuronCores (64 rows each).
Per core, per step t (h kept transposed: H on partitions):
    a = relu(tanh(h))                       (ACT + DVE)
    delta = 0.1*(a @ W_rec + x_t @ W_in + bias)   (PE: a-chunks stationary,
                                             pre-scaled W streams; x via K=3
                                             matmul with a ones-row for bias)
    h' = 0.9*h + delta                      (PE transposes delta back to
                                             T-layout, fused DVE update)
    y_t = h' @ W_out                        (batched every 25 steps from the
                                             h history buffer, PSUM->DRAM)
"""

import sys

sys.path.insert(0, "/opt/trn_rl_repo")

import numpy as np

ALPHA = 0.1
T, N, H, DIN, DOUT, INIT = 1000, 512, 512, 2, 2, 2
NCORES = 8
NS = N // NCORES          # 64 batch rows per core
WIN = 25                  # h-history window (steps)
BODY = 2 * WIN            # steps per For_i body (ping-pong windows)
NK = H // 128             # 4 H-chunks


def _build_nc(t_total=T, reps=1):
    import concourse.bass as bass
    import concourse.mybir as mybir
    from concourse import bacc
    from concourse.tile import TileContext
    from concourse.masks import make_identity
    from concourse.bass import ds

    fp32 = mybir.dt.float32
    nc = bacc.Bacc("TRN2", target_bir_lowering=False, debug=False,
                   num_devices=NCORES)

    # -------- DRAM I/O (per core) --------
    wrec_d = nc.dram_tensor("wrec", [NK, 128, H], fp32, kind="ExternalInput").ap()
    win3_d = nc.dram_tensor("win3", [DIN + 1, H], fp32, kind="ExternalInput").ap()
    wout_d = nc.dram_tensor("wout", [NK, 128, DOUT], fp32, kind="ExternalInput").ap()
    fcw3_d = nc.dram_tensor("fcw3", [INIT + 1, H], fp32, kind="ExternalInput").ap()
    init3_d = nc.dram_tensor("init3", [INIT + 1, NS], fp32, kind="ExternalInput").ap()
    xt_d = nc.dram_tensor("xt", [DIN + 1, t_total * NS], fp32, kind="ExternalInput").ap()
    y_d = nc.dram_tensor("y", [DOUT, t_total * NS], fp32, kind="ExternalOutput").ap()

    with TileContext(nc) as tc:
        with (
            tc.tile_pool(name="wpool", bufs=1) as wpool,
            tc.tile_pool(name="hpool", bufs=1) as hpool,
            tc.tile_pool(name="apool", bufs=2) as apool,
            tc.tile_pool(name="dpool", bufs=2) as dpool,
            tc.tile_pool(name="xpool", bufs=2) as xpool,
            tc.tile_pool(name="p1", bufs=2, space="PSUM") as p1pool,
            tc.tile_pool(name="p3", bufs=2, space="PSUM") as p3pool,
            tc.tile_pool(name="py", bufs=2, space="PSUM") as pypool,
        ):
            # -------- persistent SBUF --------
            wrec_sb = wpool.tile([128, NK, H], fp32)       # 0.1*W_rec chunks
            win3_sb = wpool.tile([DIN + 1, H], fp32)       # 0.1*[W_in; bias]
            wout_sb = wpool.tile([128, NK, DOUT], fp32)    # W_out chunks
            _ = bass  # keep import
            fcw3_sb = wpool.tile([INIT + 1, H], fp32)      # [fc_w.T; fc_b]
            init3_sb = wpool.tile([INIT + 1, NS], fp32)    # [initdir.T; ones]
            ident = wpool.tile([NS, NS], fp32)             # 64x64 identity
            hist_a = hpool.tile([128, NK, WIN * NS], fp32)
            hist_b = hpool.tile([128, NK, WIN * NS], fp32)
            hist = [hist_a, hist_b]

            for k in range(NK):
                nc.sync.dma_start(out=wrec_sb[:, k, :], in_=wrec_d[k])
                nc.sync.dma_start(out=wout_sb[:, k, :], in_=wout_d[k])
            nc.sync.dma_start(out=win3_sb[:], in_=win3_d)
            nc.sync.dma_start(out=fcw3_sb[:], in_=fcw3_d)
            nc.sync.dma_start(out=init3_sb[:], in_=init3_d)
            make_identity(nc, ident[:])

            # -------- h0 = fc(initdir), written to hist[1] slice WIN-1 --------
            ph0 = p3pool.tile([128, NK * NS], fp32)
            for m in range(NK):
                nc.tensor.matmul(ph0[:, m * NS:(m + 1) * NS],
                                 fcw3_sb[:, m * 128:(m + 1) * 128],
                                 init3_sb[:], start=True, stop=True)
            nc.scalar.copy(out=hist[1][:, :, (WIN - 1) * NS: WIN * NS],
                           in_=ph0[:].rearrange("p (k n) -> p k n", k=NK))

            # -------- time loop (outer reps loop is for benchmarking) --------
            with tc.For_i(0, reps, 1) as _rep, tc.For_i(0, t_total, BODY) as iv:
                xbuf = xpool.tile([DIN + 1, BODY * NS], fp32)
                nc.sync.dma_start(out=xbuf[:], in_=xt_d[:, ds(iv * NS, BODY * NS)])

                for w in range(2):
                    hc, hp = hist[w], hist[1 - w]
                    for s in range(WIN):
                        prev = (hp[:, :, (WIN - 1) * NS: WIN * NS] if s == 0
                                else hc[:, :, (s - 1) * NS: s * NS])
                        cur = hc[:, :, s * NS:(s + 1) * NS]
                        import concourse.mybir as _mb
                        a_t = apool.tile([128, NK * NS], fp32)
                        av = a_t[:].rearrange("p (k n) -> p k n", k=NK)
                        nc.scalar.activation(av, prev,
                                             _mb.ActivationFunctionType.Tanh)
                        nc.vector.tensor_scalar_max(a_t[:], a_t[:], 0.0)

                        psum1 = p1pool.tile([NS, H], fp32)
                        for k in range(NK):
                            nc.tensor.matmul(psum1[:],
                                             a_t[:, k * NS:(k + 1) * NS],
                                             wrec_sb[:, k, :],
                                             start=(k == 0), stop=False)
                        xcol = (w * WIN + s) * NS
                        nc.tensor.matmul(psum1[:], xbuf[:, xcol:xcol + NS],
                                         win3_sb[:], start=False, stop=True)

                        d_t = dpool.tile([NS, H], fp32)
                        nc.scalar.copy(out=d_t[:], in_=psum1[:])

                        psum3 = p3pool.tile([128, NK * NS], fp32)
                        for b in range(NK):
                            nc.tensor.transpose(psum3[:, b * NS:(b + 1) * NS],
                                                d_t[:, b * 128:(b + 1) * 128],
                                                ident[:])
                        nc.vector.scalar_tensor_tensor(
                            out=cur, in0=prev, scalar=1.0 - ALPHA,
                            in1=psum3[:].rearrange("p (k n) -> p k n", k=NK),
                            op0=_mb.AluOpType.mult, op1=_mb.AluOpType.add)

                    # y flush for this window: yT = W_out.T @ hT (batched)
                    QW = WIN * NS // 4  # 400 columns per quarter
                    for q in range(4):
                        py = pypool.tile([DOUT, QW], fp32)
                        for k in range(NK):
                            nc.tensor.matmul(py[:], wout_sb[:, k, :],
                                             hc[:, k, q * QW:(q + 1) * QW],
                                             start=(k == 0), stop=(k == NK - 1))
                        ysb = dpool.tile([DOUT, QW], fp32, tag="ysb")
                        nc.vector.tensor_copy(ysb[:], py[:])
                        nc.sync.dma_start(
                            out=y_d[:, ds(iv * NS + w * WIN * NS + q * QW, QW)],
                            in_=ysb[:])

    nc.compile()
    return nc


_NC_CACHE = {}


def _get_nc():
    if "nc" not in _NC_CACHE:
        _NC_CACHE["nc"] = _build_nc()
    return _NC_CACHE["nc"]


def _prep_in_maps(initdir, velocities, fc_w, fc_b, W_in, W_rec, W_out, bias):
    initdir = np.asarray(initdir, np.float32)
    velocities = np.asarray(velocities, np.float32)
    fc_w = np.asarray(fc_w, np.float32)
    fc_b = np.asarray(fc_b, np.float32)
    W_in = np.asarray(W_in, np.float32)
    W_rec = np.asarray(W_rec, np.float32)
    W_out = np.asarray(W_out, np.float32)
    bias = np.asarray(bias, np.float32)

    # host-side weight prep (shared across cores)
    wrec = (ALPHA * W_rec).reshape(NK, 128, H)
    win3 = ALPHA * np.concatenate([W_in, bias[None, :]], axis=0)     # (3, H)
    wout = W_out.reshape(NK, 128, DOUT)
    fcw3 = np.concatenate([fc_w.T, fc_b[None, :]], axis=0)           # (3, H)

    in_maps = []
    for c in range(NCORES):
        sl = slice(c * NS, (c + 1) * NS)
        init3 = np.concatenate([initdir[sl].T,
                                np.ones((1, NS), np.float32)], axis=0)
        # xt[p, t*NS+n] = velocities[t, c*NS+n, p]; row DIN = ones
        xs = velocities[:, sl, :]                                    # (T, NS, 2)
        xt = np.empty((DIN + 1, T * NS), np.float32)
        xt[:DIN] = xs.transpose(2, 0, 1).reshape(DIN, T * NS)
        xt[DIN] = 1.0
        in_maps.append({
            "wrec": np.ascontiguousarray(wrec),
            "win3": np.ascontiguousarray(win3),
            "wout": np.ascontiguousarray(wout),
            "fcw3": np.ascontiguousarray(fcw3),
            "init3": np.ascontiguousarray(init3),
            "xt": xt,
        })
    return in_maps


def kernel(initdir, velocities, fc_w, fc_b, W_in, W_rec, W_out, bias):
    from concourse.bass_utils import run_bass_kernel_spmd

    in_maps = _prep_in_maps(initdir, velocities, fc_w, fc_b, W_in, W_rec,
                            W_out, bias)
    nc = _get_nc()
    res = run_bass_kernel_spmd(nc, in_maps, list(range(NCORES)))

    out = np.empty((T, N, DOUT), np.float32)
    for c in range(NCORES):
        yt = res.results[c]["y"]                                     # (2, T*NS)
        out[:, c * NS:(c + 1) * NS, :] = (
            yt.reshape(DOUT, T, NS).transpose(1, 2, 0))
    return out


# revision 11
# speedup vs baseline: 83.1359x; 1.7355x over previous
"""ContinuousTimeRNN Trainium2 kernel.

Data-parallel over batch N=512 across 8 NeuronCores (64 rows each).
Per core, per step t (h kept transposed: H on partitions):
    a = relu(tanh(h))                       (ACT + DVE)
    delta = 0.1*(a @ W_rec + x_t @ W_in + bias)   (PE: a-chunks stationary,
                                             pre-scaled W streams; x via K=3
                                             matmul with a ones-row for bias)
    h' = 0.9*h + delta                      (PE transposes delta back to
                                             T-layout, fused DVE update)
    y_t = h' @ W_out                        (batched every 25 steps from the
                                             h history buffer, PSUM->DRAM)
"""

import sys

sys.path.insert(0, "/opt/trn_rl_repo")

import numpy as np

ALPHA = 0.1
T, N, H, DIN, DOUT, INIT = 1000, 512, 512, 2, 2, 2
NCORES = 8
NS = N // NCORES          # 64 batch rows per core
WIN = 25                  # h-history window (steps)
BODY = 2 * WIN            # steps per For_i body (ping-pong windows)
NK = H // 128             # 4 H-chunks


def _build_nc(t_total=T, reps=1):
    import concourse.bass as bass
    import concourse.mybir as mybir
    from concourse import bacc
    from concourse.tile import TileContext
    from concourse.masks import make_identity
    from concourse.bass import ds

    fp32 = mybir.dt.float32
    fp32r = mybir.dt.float32r

    def r(ap):
        return ap.bitcast(fp32r)
    nc = bacc.Bacc("TRN2", target_bir_lowering=False, debug=False,
                   num_devices=NCORES)

    # -------- DRAM I/O (per core) --------
    wrec_d = nc.dram_tensor("wrec", [NK, 128, H], fp32r, kind="ExternalInput").ap()
    win3_d = nc.dram_tensor("win3", [DIN + 1, H], fp32r, kind="ExternalInput").ap()
    wout_d = nc.dram_tensor("wout", [NK, 128, DOUT], fp32, kind="ExternalInput").ap()
    fcw3_d = nc.dram_tensor("fcw3", [INIT + 1, H], fp32, kind="ExternalInput").ap()
    init3_d = nc.dram_tensor("init3", [INIT + 1, NS], fp32, kind="ExternalInput").ap()
    xt_d = nc.dram_tensor("xt", [DIN + 1, t_total * NS], fp32r, kind="ExternalInput").ap()
    y_d = nc.dram_tensor("y", [DOUT, t_total * NS], fp32, kind="ExternalOutput").ap()

    with TileContext(nc) as tc:
        with (
            tc.tile_pool(name="wpool", bufs=1) as wpool,
            tc.tile_pool(name="hpool", bufs=1) as hpool,
            tc.tile_pool(name="apool", bufs=2) as apool,
            tc.tile_pool(name="dpool", bufs=2) as dpool,
            tc.tile_pool(name="xpool", bufs=2) as xpool,
            tc.tile_pool(name="p1", bufs=2, space="PSUM") as p1pool,
            tc.tile_pool(name="p3", bufs=2, space="PSUM") as p3pool,
            tc.tile_pool(name="py", bufs=2, space="PSUM") as pypool,
        ):
            # -------- persistent SBUF --------
            wrec_sb = wpool.tile([128, NK, H], fp32r)       # 0.1*W_rec chunks
            win3_sb = wpool.tile([DIN + 1, H], fp32r)       # 0.1*[W_in; bias]
            wout_sb = wpool.tile([128, NK, DOUT], fp32)    # W_out chunks
            _ = bass  # keep import
            fcw3_sb = wpool.tile([INIT + 1, H], fp32)      # [fc_w.T; fc_b]
            init3_sb = wpool.tile([INIT + 1, NS], fp32)    # [initdir.T; ones]
            ident = wpool.tile([NS, NS], fp32)             # 64x64 identity
            hist_a = hpool.tile([128, NK, WIN * NS], fp32)
            hist_b = hpool.tile([128, NK, WIN * NS], fp32)
            hist = [hist_a, hist_b]

            for k in range(NK):
                nc.sync.dma_start(out=wrec_sb[:, k, :], in_=wrec_d[k])
                nc.sync.dma_start(out=wout_sb[:, k, :], in_=wout_d[k])
            nc.sync.dma_start(out=win3_sb[:], in_=win3_d)
            nc.sync.dma_start(out=fcw3_sb[:], in_=fcw3_d)
            nc.sync.dma_start(out=init3_sb[:], in_=init3_d)
            make_identity(nc, ident[:])

            # -------- h0 = fc(initdir), written to hist[1] slice WIN-1 --------
            ph0 = p3pool.tile([128, NK * NS], fp32)
            for m in range(NK):
                nc.tensor.matmul(ph0[:, m * NS:(m + 1) * NS],
                                 fcw3_sb[:, m * 128:(m + 1) * 128],
                                 init3_sb[:], start=True, stop=True)
            nc.scalar.copy(out=hist[1][:, :, (WIN - 1) * NS: WIN * NS],
                           in_=ph0[:].rearrange("p (k n) -> p k n", k=NK))

            # -------- time loop (outer reps loop is for benchmarking) --------
            with tc.For_i(0, reps, 1) as _rep, tc.For_i(0, t_total, BODY) as iv:
                xbuf = xpool.tile([DIN + 1, BODY * NS], fp32r)
                nc.sync.dma_start(out=xbuf[:], in_=xt_d[:, ds(iv * NS, BODY * NS)])

                for w in range(2):
                    hc, hp = hist[w], hist[1 - w]
                    for s in range(WIN):
                        prev = (hp[:, :, (WIN - 1) * NS: WIN * NS] if s == 0
                                else hc[:, :, (s - 1) * NS: s * NS])
                        cur = hc[:, :, s * NS:(s + 1) * NS]
                        import concourse.mybir as _mb
                        a_t = apool.tile([128, NK * NS], fp32r)
                        rbuf = apool.tile([128, NK * NS], fp32, tag="rbuf")
                        rv = rbuf[:].rearrange("p (k n) -> p k n", k=NK)
                        nc.vector.tensor_scalar_max(rv, prev, 0.0)
                        nc.scalar.activation(a_t[:], rbuf[:],
                                             _mb.ActivationFunctionType.Tanh)

                        psum1 = p1pool.tile([NS, H], fp32)
                        for k in range(NK):
                            nc.tensor.matmul(psum1[:],
                                             a_t[:, k * NS:(k + 1) * NS],
                                             wrec_sb[:, k, :],
                                             start=(k == 0), stop=False)
                        xcol = (w * WIN + s) * NS
                        nc.tensor.matmul(psum1[:], xbuf[:, xcol:xcol + NS],
                                         win3_sb[:], start=False, stop=True)

                        d_t = dpool.tile([NS, H], fp32)
                        nc.scalar.copy(out=d_t[:, 0:H // 2], in_=psum1[:, 0:H // 2])
                        nc.vector.tensor_copy(d_t[:, H // 2:H], psum1[:, H // 2:H])

                        psum3 = p3pool.tile([128, NK * NS], fp32)
                        for b in range(NK):
                            nc.tensor.transpose(psum3[:, b * NS:(b + 1) * NS],
                                                d_t[:, b * 128:(b + 1) * 128],
                                                ident[:])
                        nc.vector.scalar_tensor_tensor(
                            out=cur, in0=prev, scalar=1.0 - ALPHA,
                            in1=psum3[:].rearrange("p (k n) -> p k n", k=NK),
                            op0=_mb.AluOpType.mult, op1=_mb.AluOpType.add)

                    # y flush for this window: yT = W_out.T @ hT (batched)
                    QW = WIN * NS // 4  # 400 columns per quarter
                    for q in range(4):
                        py = pypool.tile([DOUT, QW], fp32)
                        for k in range(NK):
                            nc.tensor.matmul(py[:], wout_sb[:, k, :],
                                             hc[:, k, q * QW:(q + 1) * QW],
                                             start=(k == 0), stop=(k == NK - 1))
                        ysb = dpool.tile([DOUT, QW], fp32, tag="ysb")
                        nc.vector.tensor_copy(ysb[:], py[:])
                        nc.sync.dma_start(
                            out=y_d[:, ds(iv * NS + w * WIN * NS + q * QW, QW)],
                            in_=ysb[:])

    nc.compile()
    return nc


_NC_CACHE = {}


def _get_nc():
    if "nc" not in _NC_CACHE:
        _NC_CACHE["nc"] = _build_nc()
    return _NC_CACHE["nc"]


def _prep_in_maps(initdir, velocities, fc_w, fc_b, W_in, W_rec, W_out, bias):
    initdir = np.asarray(initdir, np.float32)
    velocities = np.asarray(velocities, np.float32)
    fc_w = np.asarray(fc_w, np.float32)
    fc_b = np.asarray(fc_b, np.float32)
    W_in = np.asarray(W_in, np.float32)
    W_rec = np.asarray(W_rec, np.float32)
    W_out = np.asarray(W_out, np.float32)
    bias = np.asarray(bias, np.float32)

    # host-side weight prep (shared across cores)
    wrec = (ALPHA * W_rec).reshape(NK, 128, H)
    win3 = ALPHA * np.concatenate([W_in, bias[None, :]], axis=0)     # (3, H)
    wout = W_out.reshape(NK, 128, DOUT)
    fcw3 = np.concatenate([fc_w.T, fc_b[None, :]], axis=0)           # (3, H)

    in_maps = []
    for c in range(NCORES):
        sl = slice(c * NS, (c + 1) * NS)
        init3 = np.concatenate([initdir[sl].T,
                                np.ones((1, NS), np.float32)], axis=0)
        # xt[p, t*NS+n] = velocities[t, c*NS+n, p]; row DIN = ones
        xs = velocities[:, sl, :]                                    # (T, NS, 2)
        xt = np.empty((DIN + 1, T * NS), np.float32)
        xt[:DIN] = xs.transpose(2, 0, 1).reshape(DIN, T * NS)
        xt[DIN] = 1.0
        in_maps.append({
            "wrec": np.ascontiguousarray(wrec),
            "win3": np.ascontiguousarray(win3),
            "wout": np.ascontiguousarray(wout),
            "fcw3": np.ascontiguousarray(fcw3),
            "init3": np.ascontiguousarray(init3),
            "xt": xt,
        })
    return in_maps


def kernel(initdir, velocities, fc_w, fc_b, W_in, W_rec, W_out, bias):
    from concourse.bass_utils import run_bass_kernel_spmd

    in_maps = _prep_in_maps(initdir, velocities, fc_w, fc_b, W_in, W_rec,
                            W_out, bias)
    nc = _get_nc()
    res = run_bass_kernel_spmd(nc, in_maps, list(range(NCORES)))

    out = np.empty((T, N, DOUT), np.float32)
    for c in range(NCORES):
        yt = res.results[c]["y"]                                     # (2, T*NS)
        out[:, c * NS:(c + 1) * NS, :] = (
            yt.reshape(DOUT, T, NS).transpose(1, 2, 0))
    return out
